# revision 11
# baseline (speedup 1.0000x reference)
"""Trainium2 Bass kernel for nn_MultiHeadAttention_25031069401563.

Sharding: 8 cores = (batch b in {0,1}) x (kv-head group g in {0..3}).
Each core computes, for its batch and its kv group (4 query heads, 1 kv head):
  Q/K/V projections, QK-RMSNorm (folded), RoPE, causal attention, and the
  partial o_proj against its 512-column slice of Wo.  The host sums the 4
  partial outputs per batch (tensor-parallel all-reduce done on host).

Device algorithm (per core), all matmuls bf16 x bf16 -> fp32 PSUM:
  phase 1: q = x @ WqT, kv = x @ [WkT|WvT] accumulated over H tiles;
           sum-of-squares via ScalarE Square+accum_out; Q normalized and
           roped (norm weights folded into host cos/sin tables); K roped raw
           (its RMS scale is folded into the exp() scale later); Q/K
           PE-transposed to [head_dim, token] layout.
  phase 2: per (q-chunk, head): S^T tiles = K^T-tile.T @ Q^T-chunk (one
           matmul, full head_dim contraction), exp on ScalarE with
           per-partition scale = k-token RMS scale / sqrt(head_dim)
           (softmax max-subtraction is safely skipped: |s| <= sqrt(128)),
           causal masking of diagonal tiles via GpSimd affine_select,
           denominator via ones-vector matmul, attn^T accumulated as
           V-tile.T @ E; normalize with reciprocal_approx_fast +
           partition_broadcast; o_proj directly from attn^T slices.
"""

import functools

import ml_dtypes
import numpy as np

H = 2048
S = 2048
HD = 128
NH = 16
NKV = 4
EPS = 1e-6
ROPE_BASE = 10000.0

P = 128
NT = S // P            # 16 token tiles
NHT = H // P           # 16 hidden tiles
QC = 512               # q-chunk width (free dim of S^T / attn^T tiles)
NQC = S // QC          # 4
NQH = NH // NKV        # 4 query heads per core
NCORES = 8
B = 2

BF16 = ml_dtypes.bfloat16


# ---------------------------------------------------------------- host prep

def _rope_tables():
    inv_freq = 1.0 / (ROPE_BASE ** (np.arange(0, HD, 2, dtype=np.float32) / HD))
    pos = np.arange(S, dtype=np.float32)
    ang = pos[:, None] * inv_freq[None, :]
    emb = np.concatenate([ang, ang], axis=-1)  # [S, HD]
    return np.cos(emb).astype(np.float32), np.sin(emb).astype(np.float32)


def _fold_tables(cos, sin, w):
    """Fold the RMSNorm elementwise weight into the rope tables.

    Device computes: out[i] = x[i]*cosw[i] + x[(i+64)%128]*sinw[i],
    which must equal (w*x)[i]*cos[i] + rotate_half(w*x)[i]*sin[i]."""
    w = w.astype(np.float32)
    cosw = cos * w[None, :]
    w_rot = np.concatenate([w[64:], w[:64]])
    sgn = np.concatenate([-np.ones(64, np.float32), np.ones(64, np.float32)])
    sinw = sin * (w_rot * sgn)[None, :]
    return cosw, sinw


def _core_inputs(hidden_states, Wq, Wk, Wv, Wo, q_norm_w, k_norm_w):
    cos, sin = _rope_tables()
    cosq, sinq = _fold_tables(cos, sin, np.asarray(q_norm_w))
    cosk, sink = _fold_tables(cos, sin, np.asarray(k_norm_w))
    tables = {
        "cosq": np.ascontiguousarray(cosq.astype(BF16)),
        "sinq": np.ascontiguousarray(sinq.astype(BF16)),
        "cosk": np.ascontiguousarray(cosk.astype(BF16)),
        "sink": np.ascontiguousarray(sink.astype(BF16)),
    }
    x = np.asarray(hidden_states, np.float32)
    Wq = np.asarray(Wq, np.float32)
    Wk = np.asarray(Wk, np.float32)
    Wv = np.asarray(Wv, np.float32)
    Wo = np.asarray(Wo, np.float32)

    in_maps = []
    for core in range(NCORES):
        b, g = core // NKV, core % NKV
        wkv = np.concatenate(
            [Wk[HD * g:HD * (g + 1), :].T, Wv[HD * g:HD * (g + 1), :].T], axis=1)
        m = {
            "xT": np.ascontiguousarray(x[b].T.astype(BF16)),
            "wqT": np.ascontiguousarray(
                Wq[512 * g:512 * (g + 1), :].T.astype(BF16)),
            "wkvT": np.ascontiguousarray(wkv.astype(BF16)),
            "wo": np.ascontiguousarray(
                Wo[:, 512 * g:512 * (g + 1)].T.astype(BF16)),
            **tables,
        }
        in_maps.append(m)
    return in_maps


# ------------------------------------------------------------- device build

def _emit_body(nc, tc, mybir, bass, res, work, psum):
    """Emit one full forward pass. `res` holds the resident SBUF tiles."""
    f32 = mybir.dt.float32
    bf = mybir.dt.bfloat16
    Alu = mybir.AluOpType
    Act = mybir.ActivationFunctionType

    d = nc.dram_aps  # dict of dram APs, stashed by _build

    # ---- input DMAs (split so compute can start early)
    for ht in range(NHT):
        nc.sync.dma_start(out=res["xT"][:, ht, :],
                          in_=d["xT"][ht * P:(ht + 1) * P, :])
        nc.sync.dma_start(out=res["wq"][:, ht, :],
                          in_=d["wqT"][ht * P:(ht + 1) * P, :])
        nc.sync.dma_start(out=res["wkv"][:, ht, :],
                          in_=d["wkvT"][ht * P:(ht + 1) * P, :])
    for ft in range(NQH):
        nc.sync.dma_start(out=res["wo"][:, ft, :],
                          in_=d["wo"][ft * P:(ft + 1) * P, :])
    for name in ("cosq", "sinq", "cosk", "sink"):
        nc.sync.dma_start(
            out=res[name],
            in_=d[name].rearrange("(tt p) hd -> p tt hd", p=P))

    from concourse.masks import make_identity
    make_identity(nc, res["ident"])
    nc.vector.memset(res["ones"], 1.0)
    nc.vector.memset(res["eps_q"], EPS)
    nc.vector.memset(res["eps_k"], HD * EPS)

    def bcast_heads(ap2d, n):
        return bass.AP(tensor=ap2d.tensor, offset=ap2d.offset,
                       ap=[ap2d.ap[0], [0, n], *ap2d.ap[1:]])

    def rot_view(ap, nh):
        """[P, nh, HD] view reading each head's halves swapped."""
        a = ap.ap
        assert a[-1][0] == 1 and a[-1][1] == HD
        head = [] if nh == 1 else [a[-2]]
        return bass.AP(tensor=ap.tensor, offset=ap.offset + 64,
                       ap=[a[0], *head, [-64, 2], [1, 64]])

    # ================= phase 1: projections, norms, rope, transposes
    for tt in range(NT):
        ts = slice(tt * P, (tt + 1) * P)
        qp = psum.tile([P, 4 * HD], f32, tag="ps_a")
        kvp = psum.tile([P, 2 * HD], f32, tag="ps_b")
        for ht in range(NHT):
            lhs = res["xT"][:, ht, ts]
            nc.tensor.matmul(qp, lhsT=lhs, rhs=res["wq"][:, ht, :],
                             start=(ht == 0), stop=(ht == NHT - 1))
            nc.tensor.matmul(kvp, lhsT=lhs, rhs=res["wkv"][:, ht, :],
                             start=(ht == 0), stop=(ht == NHT - 1))
        kp = kvp[:, 0:HD]
        vp = kvp[:, HD:2 * HD]
        # V straight to bf16 SBUF
        nc.vector.tensor_copy(res["v"][:, tt, :], vp)

        # sum of squares for q (per head) and k, via Square + accum_out
        sums = work.tile([P, 5], f32, tag="sums")
        scr = work.tile([P, HD], bf, tag="scr")
        for h in range(NQH):
            nc.scalar.activation(scr, qp[:, h * HD:(h + 1) * HD], Act.Square,
                                 accum_out=sums[:, h:h + 1])
        nc.scalar.activation(scr, kp, Act.Square, accum_out=sums[:, 4:5])
        sc = work.tile([P, 5], f32, tag="sc")
        nc.scalar.activation(sc[:, 0:4], sums[:, 0:4], Act.Sqrt,
                             scale=1.0 / HD, bias=res["eps_q"])
        nc.scalar.activation(sc[:, 4:5], sums[:, 4:5], Act.Sqrt,
                             scale=1.0, bias=res["eps_k"])
        rc = work.tile([P, 5], f32, tag="rc")
        nc.vector.reciprocal(rc, sc)
        nc.vector.tensor_copy(res["sk"][:, tt:tt + 1], rc[:, 4:5])

        # Q: normalize (per head) -> bf16, then rope
        qn = work.tile([P, NQH, HD], bf, tag="qn")
        for h in range(NQH):
            nc.vector.tensor_scalar_mul(qn[:, h, :], qp[:, h * HD:(h + 1) * HD],
                                        rc[:, h:h + 1])
        t1 = work.tile([P, NQH, HD], bf, tag="t1")
        t2 = work.tile([P, NQH, HD], bf, tag="t2")
        qr = work.tile([P, NQH, HD], bf, tag="qr")
        nc.vector.tensor_tensor(t1, qn, bcast_heads(res["cosq"][:, tt, :], NQH),
                                Alu.mult)
        nc.vector.tensor_tensor(t2, rot_view(qn, NQH),
                                bcast_heads(res["sinq"][:, tt, :], NQH),
                                Alu.mult)
        nc.vector.tensor_tensor(qr, t1, t2, Alu.add)

        # K: rope raw (RMS scale folded into exp scale later)
        k1 = work.tile([P, HD], bf, tag="k1")
        k2 = work.tile([P, HD], bf, tag="k2")
        kr = work.tile([P, HD], bf, tag="kr")
        nc.vector.tensor_tensor(k1, kp, res["cosk"][:, tt, :], Alu.mult)
        nc.vector.tensor_tensor(k2, rot_view(kp, 1), res["sink"][:, tt, :],
                                Alu.mult)
        nc.vector.tensor_tensor(kr, k1, k2, Alu.add)

        # transposes -> [hd, token] layout
        for h in range(NQH):
            tp = psum.tile([P, P], bf, tag="ps_c")
            nc.tensor.transpose(tp, qr[:, h, :], res["ident"])
            nc.vector.tensor_copy(res["qT"][:, h, ts], tp)
        tp = psum.tile([P, P], bf, tag="ps_c")
        nc.tensor.transpose(tp, kr, res["ident"])
        nc.vector.tensor_copy(res["kT"][:, ts], tp)

    # ================= phase 2: attention + o_proj
    for qc in range(NQC):
        qs = slice(qc * QC, (qc + 1) * QC)
        attnT = work.tile([P, NQH, QC], bf, tag="attnT")
        for h in range(NQH):
            av = psum.tile([P, QC], f32, tag="ps_b")
            dn = psum.tile([1, QC], f32, tag="ps_c")
            nkt = 4 * qc + 4
            for kt in range(nkt):
                st = psum.tile([P, QC], f32, tag="ps_a")
                nc.tensor.matmul(st, lhsT=res["kT"][:, kt * P:(kt + 1) * P],
                                 rhs=res["qT"][:, h, qs],
                                 start=True, stop=True)
                e = work.tile([P, QC], bf, tag="e")
                nc.scalar.activation(e, st, Act.Exp,
                                     scale=res["sk"][:, kt:kt + 1])
                if kt >= 4 * qc:  # tile straddles the causal diagonal
                    nc.gpsimd.affine_select(
                        out=e, in_=e, compare_op=Alu.is_ge, fill=0.0,
                        base=qc * QC - kt * P, pattern=[[1, QC]],
                        channel_multiplier=-1)
                nc.tensor.matmul(dn, lhsT=res["ones"], rhs=e,
                                 start=(kt == 0), stop=(kt == nkt - 1))
                nc.tensor.matmul(av, lhsT=res["v"][:, kt, :], rhs=e,
                                 start=(kt == 0), stop=(kt == nkt - 1))
            dcp = work.tile([1, QC], f32, tag="dcp")
            nc.vector.tensor_copy(dcp, dn)
            rcp = work.tile([1, QC], f32, tag="rcp")
            nc.vector.reciprocal_approx_fast(rcp, dcp)
            bc = work.tile([P, QC], f32, tag="bc")
            nc.gpsimd.partition_broadcast(bc, rcp)
            nc.vector.tensor_tensor(attnT[:, h, :], av, bc, Alu.mult)

        # o_proj for this q-chunk
        for t4 in range(QC // P):
            tt = qc * (QC // P) + t4
            for hc in range(H // 512):
                op = psum.tile([P, 512], f32, tag="ps_d")
                for ft in range(NQH):
                    nc.tensor.matmul(
                        op, lhsT=attnT[:, ft, t4 * P:(t4 + 1) * P],
                        rhs=res["wo"][:, ft, hc * 512:(hc + 1) * 512],
                        start=(ft == 0), stop=(ft == NQH - 1))
                ost = work.tile([P, 512], f32, tag="ost")
                nc.vector.tensor_copy(ost, op)
                nc.sync.dma_start(
                    out=d["out"][tt * P:(tt + 1) * P, hc * 512:(hc + 1) * 512],
                    in_=ost)


def _build(with_loop=False):
    import concourse.bass as bass
    import concourse.mybir as mybir
    import concourse.tile as tile
    from concourse import bacc

    f32 = mybir.dt.float32
    bf = mybir.dt.bfloat16

    nc = bacc.Bacc("TRN2", target_bir_lowering=False, debug=False)
    d = {}
    d["xT"] = nc.dram_tensor("xT", [H, S], bf, kind="ExternalInput").ap()
    d["wqT"] = nc.dram_tensor("wqT", [H, 4 * HD], bf, kind="ExternalInput").ap()
    d["wkvT"] = nc.dram_tensor("wkvT", [H, 2 * HD], bf,
                               kind="ExternalInput").ap()
    d["wo"] = nc.dram_tensor("wo", [4 * HD, H], bf, kind="ExternalInput").ap()
    for name in ("cosq", "sinq", "cosk", "sink"):
        d[name] = nc.dram_tensor(name, [S, HD], bf, kind="ExternalInput").ap()
    d["out"] = nc.dram_tensor("out", [S, H], f32, kind="ExternalOutput").ap()
    nc.dram_aps = d

    with tile.TileContext(nc) as tc:
        from contextlib import ExitStack
        with ExitStack() as stk:
            const = stk.enter_context(tc.tile_pool(name="const", bufs=1))
            work = stk.enter_context(tc.tile_pool(name="work", bufs=3))
            psum = stk.enter_context(
                tc.tile_pool(name="psum", bufs=2, space="PSUM"))

            shapes = {
                "xT": ([P, NHT, S], bf),
                "wq": ([P, NHT, 4 * HD], bf),
                "wkv": ([P, NHT, 2 * HD], bf),
                "wo": ([P, NQH, H], bf),
                "cosq": ([P, NT, HD], bf),
                "sinq": ([P, NT, HD], bf),
                "cosk": ([P, NT, HD], bf),
                "sink": ([P, NT, HD], bf),
                "qT": ([P, NQH, S], bf),
                "kT": ([P, S], bf),
                "v": ([P, NT, HD], bf),
                "sk": ([P, NT], f32),
                "ident": ([P, P], bf),
                "ones": ([P, 1], bf),
                "eps_q": ([P, 1], f32),
                "eps_k": ([P, 1], f32),
            }
            res = {k: const.tile(shape, dt, tag=k, name=k)
                   for k, (shape, dt) in shapes.items()}

            if with_loop and with_loop > 1:
                with tc.For_i(0, int(with_loop)) as _i:
                    _emit_body(nc, tc, mybir, bass, res, work, psum)
            else:
                _emit_body(nc, tc, mybir, bass, res, work, psum)

    nc.compile()
    return nc


@functools.lru_cache(maxsize=4)
def _get_nc(with_loop=0):
    """with_loop: 0/1 = plain single-shot body; N>1 = body wrapped in a
    static hardware For_i loop of N iterations (for timing)."""
    return _build(with_loop=with_loop)


# ------------------------------------------------------------------ kernel

def kernel(hidden_states, attention_mask, Wq, Wk, Wv, Wo, q_norm_w, k_norm_w):
    from concourse import bass_utils

    nc = _get_nc(False)
    in_maps = _core_inputs(hidden_states, Wq, Wk, Wv, Wo, q_norm_w, k_norm_w)
    res = bass_utils.run_bass_kernel_spmd(nc, in_maps,
                                          core_ids=list(range(NCORES)))
    out = np.zeros((B, S, H), np.float32)
    for core in range(NCORES):
        out[core // NKV] += res.results[core]["out"]
    return out
